# revision 1
# baseline (speedup 1.0000x reference)
"""EncoderDecoder (GRU encoder + attention GRU decoder + log_softmax head)
as a Bass/Tile kernel for 8 Trainium2 NeuronCores.

Strategy: data-parallel over batch B=256 -> 8 shards of Bs=32. Each core runs
the full recurrence for its shard in a TRANSPOSED (feature-major) layout:
  hT [128 part = h%128, (chunk c, b)]  with h = 128c + p
so GRU matmuls are weight-stationary (lhsT = W.T tiles, rhs = hT chunks) and
all elementwise work uses 128 partitions. Precomputable terms (input-side GRU
gates, attention/combine embedding terms) are batched matmuls; per-step they
are folded into PSUM accumulation via identity-stationary matmuls. Biases are
folded as rank-1 outer-product matmuls ([1,128] x [1,32]).  Sigmoid uses the
tanh half-angle identity with 0.5-prescaled weights. Softmaxes skip
max-subtraction (logit ranges are tiny; validated < 2e-3 rel err).
The vocab head ([1024,512]@[512,10000] per core) is interleaved with the
decoder: one 128-row M-tile per 4 decoder steps, with exp+running-sum fused
into one ScalarE pass and the log-softmax subtraction fused into the
PSUM->SBUF eviction path.
"""
import os
import sys

import numpy as np

for _p in ("/opt/trn_rl_repo", "/root/.axon_site/_ro/trn_rl_repo"):
    if os.path.isdir(_p) and _p not in sys.path:
        sys.path.insert(0, _p)

import ml_dtypes
from contextlib import ExitStack

import concourse.bass as bass
import concourse.tile as tile
from concourse import bacc, mybir
from concourse.bass_utils import run_bass_kernel_spmd

BF = ml_dtypes.bfloat16
F32 = np.float32

H = 512
V = 10000
L = 32
B = 256
SOS = 1
NCORES = 8
BS = B // NCORES          # 32 rows per core
NT = L * BS               # 1024 batched rows per core
HK = H // 128             # 4 contraction chunks
NVC = 20                  # head col chunks
VC = V // NVC             # 500

bf = mybir.dt.bfloat16
f8 = mybir.dt.float8e4
f32 = mybir.dt.float32
AF = mybir.ActivationFunctionType
ALU = mybir.AluOpType
AX = mybir.AxisListType


# --------------------------------------------------------------------------
# program builder
# --------------------------------------------------------------------------

def declare_params(nc):
    p = {}

    def P(name, shape, d=bf, out=False):
        p[name] = nc.declare_dram_parameter(name, list(shape), d, isOutput=out)

    P("xenct", [H, NT])          # enc embeddings, transposed, col = 32t+b
    P("xdect", [H, NT])          # dec embeddings, transposed
    P("ewhht", [H, 3 * H])       # enc W_hh'.T (rz rows pre-scaled 0.5)
    P("ewihrzt", [H, 2 * H])     # enc W_ih_rz'.T (pre-scaled 0.5)
    P("ewihnt", [H, H])          # enc W_ih_n.T
    P("dwhht", [H, 3 * H])       # dec W_hh'.T
    P("dwiht", [H, 3 * H])       # dec W_ih'.T (rz pre-scaled)
    P("w1t", [H, H])             # comb_w[:, :H].T
    P("w2t", [H, H])             # comb_w[:, H:].T
    P("awt1", [H, L])            # attn_w[:, :H].T
    P("awht", [H, L])            # attn_w[:, H:].T
    P("outwt", [H, V])           # out_w.T
    P("outb", [128, V])          # out_b, row-replicated
    P("girzbias", [128, 2 * H])  # enc 0.5*(b_ih+b_hh)[:2H], row-replicated
    P("ebhhn", [128, H])         # enc b_hh[2H:], row-replicated
    P("ebihnt", [128, HK], f32)  # enc b_ih[2H:] wrapped per (p, chunk)
    P("dbias2", [128, 4 * H])    # dec biases, row-replicated
    P("cbt", [128, HK], f32)     # comb_b wrapped per (p, chunk)
    P("attnbrow", [128, L])      # attn_b, row-replicated
    P("i128", [128, 128])        # identity
    P("out", [NT, V], out=True)  # bf16 log-probs, row = 32t + b
    P("dbg", [128, 5 * 128], out=True)
    P("dbg2", [128, 16384], out=True)
    return p


def emit(ctx, tc, p):
    nc = tc.nc
    STAGE = int(os.environ.get("BASS_ENCDEC_STAGE", "4"))

    def mm(out, lhsT, rhs, start, stop=False, tp=None):
        nc.tensor.matmul(out, lhsT, rhs, start=start, stop=stop,
                         tile_position=tp, skip_group_check=True)

    def copy_on(use_act, out, in_):
        if use_act:
            nc.scalar.copy(out, in_)
        else:
            nc.vector.tensor_copy(out, in_)

    # ---------------- resident pools ------------------------------------
    const = ctx.enter_context(tc.tile_pool(name="const", bufs=1))
    persist = ctx.enter_context(tc.tile_pool(name="persist", bufs=1))

    def load(pool, name, shape, d=bf, src=None):
        """DMA a dram param into an SBUF tile.  [H, X] params land as
        [128, HK, X] (partition = h % 128, chunk = h // 128)."""
        t = pool.tile(list(shape), d, tag=name)
        ap = p[src or name].ap()
        if len(shape) == 3 and shape[0] == 128 and shape[1] == HK:
            ap = ap.rearrange("(k p) x -> p k x", p=128)
        nc.sync.dma_start(t[:], ap)
        return t

    # small constants / rows (persist whole kernel)
    I128 = load(const, "i128", [128, 128])
    DBIAS2 = load(const, "dbias2", [128, 4 * H])
    EBHHN = load(const, "ebhhn", [128, H])
    CBT = load(const, "cbt", [128, HK], f32)
    EBIHNT = load(const, "ebihnt", [128, HK], f32)
    ATTNBR = load(const, "attnbrow", [128, L])
    AWHT = load(const, "awht", [128, HK, L])
    OUTB = load(const, "outb", [128, V])

    # big persistent tensors
    G = persist.tile([128, HK, BS, L], bf, tag="G")        # enc outs [p,c,b,l]
    H2T = persist.tile([128, HK, L, BS], bf, tag="H2T")    # dec hiddens
    CE_B = persist.tile([128, 8, H], bf, tag="CE_B")       # comb emb, b-rows
    AWEMB_B = persist.tile([128, 8, L], bf, tag="AWEMB_B")  # attn emb, b-rows

    # shared across enc+dec loops
    psg_pool = ctx.enter_context(tc.tile_pool(name="psg", bufs=2, space="PSUM"))
    hwork = ctx.enter_context(tc.tile_pool(name="hwork", bufs=2))
    ew = ctx.enter_context(tc.tile_pool(name="ew", bufs=2))

    # ---------------- phase 1: enc inputs + batched precomputes ---------
    with tc.tile_pool(name="encw", bufs=1) as encw_p:
        with ExitStack() as s1:
            b1 = s1.enter_context(tc.tile_pool(name="batch1", bufs=1))
            XENCT = load(b1, "xenct", [128, HK, NT])
            EWIHRZT = load(b1, "ewihrzt", [128, HK, 2 * H])
            EWIHNT = load(b1, "ewihnt", [128, HK, H])
            GIRZB = load(b1, "girzbias", [128, 2 * H])
            EWHHT = load(encw_p, "ewhht", [128, HK, 3 * H])
            GIRZ_B = encw_p.tile([128, 8, 2 * H], bf, tag="GIRZ_B")
            INT_T = encw_p.tile([128, HK, L, BS], bf, tag="INT_T")

            with tc.tile_pool(name="pbat", bufs=2, space="PSUM") as pb:
                # GIRZ_B[r] = (X @ W_ih_rz'.T + bias') rows 128r..128r+128
                for r in range(8):
                    ps = pb.tile([128, 2 * H], f32, tag="pbat")
                    for n2 in range(2):
                        for k in range(HK):
                            mm(ps[:, 512 * n2:512 * (n2 + 1)],
                               XENCT[:, k, 128 * r:128 * (r + 1)],
                               EWIHRZT[:, k, 512 * n2:512 * (n2 + 1)],
                               start=(k == 0))
                        mm(ps[:, 512 * n2:512 * (n2 + 1)], I128[:, :],
                           GIRZB[:, 512 * n2:512 * (n2 + 1)],
                           start=False, stop=True)
                    copy_on(r % 2, GIRZ_B[:, r, :], ps[:])
                # INT_T[m] = (W_ihn @ X.T) + b_ihn   [p = n-dim chunk m]
                for m in range(HK):
                    ps = pb.tile([128, NT], f32, tag="pbat")
                    for n2 in range(2):
                        for k in range(HK):
                            mm(ps[:, 512 * n2:512 * (n2 + 1)],
                               EWIHNT[:, k, 128 * m:128 * (m + 1)],
                               XENCT[:, k, 512 * n2:512 * (n2 + 1)],
                               start=(k == 0), stop=(k == HK - 1))
                    nc.scalar.activation(
                        INT_T[:, m, :, :],
                        ps[:].rearrange("p (t b) -> p t b", b=BS),
                        AF.Identity, bias=EBIHNT[:, m:m + 1])

        # ---------------- phase 2: encoder recurrence -------------------
        hT = hwork.tile([128, HK, BS], bf, tag="hT")
        nc.vector.memset(hT[:], 0.0)
        nc.vector.memset(G[:].rearrange("p c b l -> p (c b l)"), 0.0)

        for t in range(L if STAGE >= 2 else 0):
            tr, tq = t % 4, t // 4
            pg = psg_pool.tile([128, 16, BS], f32, tag="pg")
            for j in range(12):
                for k in range(HK):
                    mm(pg[:, j, :], EWHHT[:, k, 128 * j:128 * (j + 1)],
                       hT[:, k, :], start=(j == 0 and k == 0))
            # fold precomputed input gates (rz): out[m,b] += girz[32tr+b, m]
            for j in range(8):
                mm(pg[:, j, :], GIRZ_B[:, tq, 128 * j:128 * (j + 1)],
                   I128[:, 32 * tr:32 * (tr + 1)], start=False, stop=True)
            # b_hh_n: lhsT row-replicated bias, rhs = any 32 identity cols
            for c in range(HK):
                mm(pg[:, 8 + c, :], EBHHN[:, 128 * c:128 * (c + 1)],
                   I128[:, 0:BS], start=False, stop=True)
            hT = gru_tail(nc, ew, hwork, pg, hT,
                          INT_T[:, :, t, :], G[:, :, :, t])
        nc.sync.dma_start(p["dbg2"].ap()[:, 0:4096],
                          G[:].rearrange("p c b l -> p (c b l)"))
        nc.sync.dma_start(p["dbg2"].ap()[:, 4096:8192],
                          INT_T[:].rearrange("p c t b -> p (c t b)"))
        nc.sync.dma_start(p["dbg2"].ap()[:, 8192:16384],
                          GIRZ_B[:].rearrange("p r g -> p (r g)"))

    nc.sync.dma_start(p["dbg"].ap()[:, 0:128], hT[:])
    # ---------------- phase 3: dec inputs + batched precomputes ---------
    decw = ctx.enter_context(tc.tile_pool(name="decw", bufs=1))
    OUTWT = load(decw, "outwt", [128, HK, V])
    DWHHT = load(decw, "dwhht", [128, HK, 3 * H])
    DWIHT = load(decw, "dwiht", [128, HK, 3 * H])
    W2T = load(decw, "w2t", [128, HK, H])

    with ExitStack() as s2:
        b2 = s2.enter_context(tc.tile_pool(name="batch2", bufs=1))
        XDECT = load(b2, "xdect", [128, HK, NT])
        W1T = load(b2, "w1t", [128, HK, H])
        AWT1 = load(b2, "awt1", [128, HK, L])
        with tc.tile_pool(name="pbat2", bufs=2, space="PSUM") as pb:
            for r in range(8):
                ps = pb.tile([128, H], f32, tag="pbat2")
                for k in range(HK):
                    mm(ps[:], XDECT[:, k, 128 * r:128 * (r + 1)],
                       W1T[:, k, :], start=(k == 0), stop=(k == HK - 1))
                copy_on(r % 2, CE_B[:, r, :], ps[:])
            for r in range(8):
                ps = pb.tile([128, L], f32, tag="pawe")
                for k in range(HK):
                    mm(ps[:], XDECT[:, k, 128 * r:128 * (r + 1)],
                       AWT1[:, k, :], start=(k == 0))
                mm(ps[:], I128[:, :], ATTNBR[:, :], start=False,
                   stop=True)
                nc.vector.tensor_copy(AWEMB_B[:, r, :], ps[:])

    # ---------------- phase 4: decoder + interleaved head ---------------
    psa_pool = ctx.enter_context(tc.tile_pool(name="psa", bufs=1, space="PSUM"))
    psx_pool = ctx.enter_context(tc.tile_pool(name="psx", bufs=1, space="PSUM"))
    psh_pool = ctx.enter_context(tc.tile_pool(name="psh", bufs=3, space="PSUM"))
    dwork = ctx.enter_context(tc.tile_pool(name="dwork", bufs=2))
    lg_pool = ctx.enter_context(tc.tile_pool(name="lgp", bufs=1))

    nc.vector.memset(H2T[:].rearrange("p c t b -> p (c t b)"), 0.0)
    for t in range(L if STAGE >= 3 else 0):
        tr, tq = t % 4, t // 4
        # ---- attention scores (b-layout: [32 b, 32 l]) ----
        pa = psa_pool.tile([BS, L], f32, tag="pa")
        for k in range(HK):
            mm(pa[:], hT[:, k, :], AWHT[:, k, :], start=(k == 0))
        mm(pa[:], I128[:, 32 * tr:32 * (tr + 1)], AWEMB_B[:, tq, :],
           start=False, stop=True)
        # ---- GRU hidden-side matmuls (only need hT) ----
        pg = psg_pool.tile([128, 16, BS], f32, tag="pg")
        for j in range(12):
            for k in range(HK):
                mm(pg[:, j, :], DWHHT[:, k, 128 * j:128 * (j + 1)],
                   hT[:, k, :], start=(j == 0 and k == 0))
        for jj in range(16):
            mm(pg[:, jj, :], DBIAS2[:, 128 * jj:128 * (jj + 1)],
               I128[:, 0:BS], start=False, stop=(8 <= jj < 12))
        # ---- softmax over l (free dim), then broadcast ----
        expb = dwork.tile([BS, L], bf, tag="expb")
        nc.scalar.activation(expb[:], pa[:], AF.Exp)
        esum = dwork.tile([BS, 1], f32, tag="esum")
        nc.vector.tensor_reduce(esum[:], expb[:], axis=AX.X, op=ALU.add)
        rr = dwork.tile([BS, 1], f32, tag="rr")
        nc.vector.reciprocal(rr[:], esum[:])
        awb = dwork.tile([BS, L], bf, tag="awb")
        nc.vector.tensor_scalar(awb[:], expb[:], rr[:, 0:1], None,
                                op0=ALU.mult)
        if t == 0:
            nc.sync.dma_start(p["dbg"].ap()[0:BS, 128:160], awb[:])
        flat = dwork.tile([1, BS * L], bf, tag="flat")
        nc.sync.dma_start(flat[0:1, :], awb[:])          # [32,32] -> [1,1024]
        awx = dwork.tile([128, BS, L], bf, tag="awx")
        nc.gpsimd.partition_broadcast(awx[:], flat[0:1, :])
        # ---- attention apply: applied.T[c] = sum_l G[c,:,l]*aw ----
        appl = dwork.tile([128, HK, BS], bf, tag="appl")
        for c in range(HK):
            tmp = ew.tile([128, BS, L], bf, tag="tmp")
            eng = nc.gpsimd if c == 3 else nc.vector
            eng.tensor_tensor(tmp[:], G[:, c, :, :], awx[:], op=ALU.mult)
            lad = tmp
            width = L
            while width > 2:
                width //= 2
                nxt = ew.tile([128, BS, width], bf, tag=f"lad{width}")
                nc.vector.tensor_tensor(nxt[:], lad[:, :, 0:width],
                                        lad[:, :, width:2 * width],
                                        op=ALU.add)
                lad = nxt
            nc.vector.tensor_tensor(appl[:, c, :], lad[:, :, 0],
                                    lad[:, :, 1], op=ALU.add)
        if t == 0:
            nc.sync.dma_start(p["dbg"].ap()[:, 256:384],
                              appl[:].rearrange("p c b -> p (c b)"))
        # ---- combine: xT = relu(W1@emb + W2@applied + b) ----
        px = psx_pool.tile([128, HK, BS], f32, tag="px")
        for m in range(HK):
            for k in range(HK):
                mm(px[:, m, :], W2T[:, k, 128 * m:128 * (m + 1)],
                   appl[:, k, :], start=(m == 0 and k == 0))
            mm(px[:, m, :], CE_B[:, tq, 128 * m:128 * (m + 1)],
               I128[:, 32 * tr:32 * (tr + 1)], start=False, stop=True)
        xT = dwork.tile([128, HK, BS], bf, tag="xT")
        for m in range(HK):
            nc.scalar.activation(xT[:, m, :], px[:, m, :], AF.Relu,
                                 bias=CBT[:, m:m + 1])
        if t == 0:
            nc.sync.dma_start(p["dbg"].ap()[:, 384:512],
                              xT[:].rearrange("p c b -> p (c b)"))
        # ---- GRU input-side matmuls ----
        for j in range(8):
            for k in range(HK):
                mm(pg[:, j, :], DWIHT[:, k, 128 * j:128 * (j + 1)],
                   xT[:, k, :], start=False, stop=(k == HK - 1))
        for j2 in range(4):
            for k in range(HK):
                mm(pg[:, 12 + j2, :],
                   DWIHT[:, k, 128 * (8 + j2):128 * (9 + j2)],
                   xT[:, k, :], start=False, stop=(k == HK - 1))
        hT = gru_tail(nc, ew, hwork, pg, hT, pg[:, 12:16, :],
                      H2T[:, :, t, :])
        if t == 0:
            nc.sync.dma_start(p["dbg"].ap()[:, 512:640],
                              hT[:].rearrange("p c b -> p (c b)"))

        # ---- head M-tile every 4 steps ----
        if t % 4 == 3 and STAGE >= 4:
            m = t // 4
            se = dwork.tile([128, NVC], f32, tag="se")
            LGT = lg_pool.tile([128, NVC, VC], bf, tag="LGT")
            for nn in range(NVC):
                ph = psh_pool.tile([128, VC], f32, tag="ph")
                for k in range(HK):
                    mm(ph[:], H2T[:, k, 4 * m:4 * (m + 1), :],
                       OUTWT[:, k, VC * nn:VC * (nn + 1)], start=(k == 0))
                mm(ph[:], I128[:, :], OUTB[:, VC * nn:VC * (nn + 1)],
                   start=False, stop=True)
                escr = dwork.tile([128, VC], bf, tag="escr")
                nc.scalar.activation(escr[:], ph[:], AF.Exp,
                                     accum_out=se[:, nn:nn + 1])
                nc.vector.tensor_copy(LGT[:, nn, :], ph[:])
            se1 = dwork.tile([128, 1], f32, tag="se1")
            nc.vector.tensor_reduce(se1[:], se[:], axis=AX.X, op=ALU.add)
            rse = dwork.tile([128, 1], f32, tag="rse")
            nc.vector.reciprocal(rse[:], se1[:])
            nlse = dwork.tile([128, 1], f32, tag="nlse")
            nc.scalar.activation(nlse[:], rse[:], AF.Ln)   # = -lse
            for nn in range(NVC):
                ob = dwork.tile([128, VC], bf, tag="ob")
                nc.scalar.activation(ob[:], LGT[:, nn, :], AF.Identity,
                                     bias=nlse[:, 0:1])
                nc.sync.dma_start(
                    p["out"].ap()[128 * m:128 * (m + 1),
                                  VC * nn:VC * (nn + 1)], ob[:])


def gru_tail(nc, ew, hwork, pg, hT, in_slice, arch_slice):
    """Elementwise GRU tail in transposed layout.

    pg: [128, 16, 32] psum. 0:8 = rz pre-activations (0.5-prescaled),
    8:12 = hidden-side n (+b_hh_n), in_slice = input-side n term.
    arch_slice: [128, HK, 32] view of G or H2T to archive the new hidden.
    """
    trz = ew.tile([128, 8, BS], bf, tag="trz")
    nc.scalar.activation(trz[:], pg[:, 0:8, :], AF.Tanh)
    rz = ew.tile([128, 8, BS], bf, tag="rz")
    nc.vector.tensor_scalar(rz[:], trz[:], 0.5, 0.5, op0=ALU.mult,
                            op1=ALU.add)
    u = ew.tile([128, HK, BS], bf, tag="u")
    nc.vector.tensor_tensor(u[:], rz[:, 0:4, :], pg[:, 8:12, :], op=ALU.mult)
    v = ew.tile([128, HK, BS], bf, tag="v")
    nc.vector.tensor_tensor(v[:], u[:], in_slice, op=ALU.add)
    n_ = ew.tile([128, HK, BS], bf, tag="n_")
    nc.scalar.activation(n_[:], v[:], AF.Tanh)
    s_ = ew.tile([128, HK, BS], bf, tag="s_")
    nc.vector.tensor_tensor(s_[:], hT[:], n_[:], op=ALU.subtract)
    w_ = ew.tile([128, HK, BS], bf, tag="w_")
    nc.vector.tensor_tensor(w_[:], rz[:, 4:8, :], s_[:], op=ALU.mult)
    hT2 = hwork.tile([128, HK, BS], bf, tag="hT")
    nc.vector.tensor_tensor(hT2[:], n_[:], w_[:], op=ALU.add)
    nc.vector.tensor_copy(arch_slice, hT2[:])
    return hT2


# --------------------------------------------------------------------------
# host-side preparation
# --------------------------------------------------------------------------

def prep_shared(inputs):
    """Weight preprocessing shared by all cores. Returns dict name->array."""
    g = lambda k: np.asarray(inputs[k], dtype=np.float32)
    ewih, ewhh = g("enc_w_ih"), g("enc_w_hh")
    ebih, ebhh = g("enc_b_ih"), g("enc_b_hh")
    dwih, dwhh = g("dec_w_ih"), g("dec_w_hh")
    dbih, dbhh = g("dec_b_ih"), g("dec_b_hh")
    attw, attb = g("attn_w"), g("attn_b")
    cw, cb = g("comb_w"), g("comb_b")
    ow, ob = g("out_w"), g("out_b")

    def scale_rz(w):  # [3H, H] -> rz rows * 0.5
        w = w.copy()
        w[:2 * H] *= 0.5
        return w

    d = {}
    d["ewhht"] = scale_rz(ewhh).T
    d["ewihrzt"] = (0.5 * ewih[:2 * H]).T
    d["ewihnt"] = ewih[2 * H:].T
    d["dwhht"] = scale_rz(dwhh).T
    d["dwiht"] = scale_rz(dwih).T
    d["w1t"] = cw[:, :H].T
    d["w2t"] = cw[:, H:].T
    d["awt1"] = attw[:, :H].T
    d["awht"] = attw[:, H:].T
    d["outwt"] = ow.T
    d["outb"] = np.tile(ob[None, :], (128, 1))
    d["girzbias"] = np.tile((0.5 * (ebih + ebhh)[:2 * H])[None, :], (128, 1))
    d["ebhhn"] = np.tile(ebhh[2 * H:][None, :], (128, 1))
    d["ebihnt"] = ebih[2 * H:].reshape(HK, 128).T.copy()
    d["dbias2"] = np.tile(np.concatenate(
        [0.5 * (dbih + dbhh)[:2 * H], dbhh[2 * H:],
         dbih[2 * H:]])[None, :], (128, 1))
    d["cbt"] = cb.reshape(HK, 128).T.copy()
    d["attnbrow"] = np.tile(attb[None, :], (128, 1))
    d["i128"] = np.eye(128, dtype=np.float32)

    out = {}
    for k, v in d.items():
        dt = F32 if k in ("ebihnt", "cbt") else (
            BF)
        out[k] = np.ascontiguousarray(v.astype(dt))
    return out


def prep_core(inputs, core):
    """Per-core embedding gathers (transposed layouts)."""
    inp = np.asarray(inputs["input_tensor"])[core * BS:(core + 1) * BS]
    tgt = np.asarray(inputs["target_tensor"])[core * BS:(core + 1) * BS]
    enc_tok = inp.T                       # [L, BS]
    dec_tok = np.empty_like(tgt.T)
    dec_tok[0] = SOS
    dec_tok[1:] = tgt.T[:-1]
    ee = np.asarray(inputs["enc_embed"], np.float32).astype(BF)
    de = np.asarray(inputs["dec_embed"], np.float32).astype(BF)
    xenc = ee[enc_tok]                    # [L, BS, H]
    xdec = de[dec_tok]
    return {
        "xenct": np.ascontiguousarray(xenc.transpose(2, 0, 1).reshape(H, NT)),
        "xdect": np.ascontiguousarray(xdec.transpose(2, 0, 1).reshape(H, NT)),
    }


_CACHE = {}


def build_program():
    if "nc" in _CACHE:
        return _CACHE["nc"]
    nc = bacc.Bacc("TRN2", target_bir_lowering=False, debug=False)
    params = declare_params(nc)
    with tile.TileContext(nc) as tc:
        with ExitStack() as ctx:
            emit(ctx, tc, params)
    nc.compile()
    _CACHE["nc"] = nc
    return nc


LAST_EXEC_NS = None
LAST_TRACE = None


def _ensure_ntff_hook():
    """Provide antenv.axon_hooks if the image lacks it (dev tracing only)."""
    try:
        from antenv.axon_hooks import get_axon_ntff_profile_hook  # noqa: F401
        return
    except ImportError:
        pass
    try:
        import types
        import antenv
        from trn_agent_boot.trn_boot import _ntff_profile_via_ctypes
        m = types.ModuleType("antenv.axon_hooks")
        state = {"h": _ntff_profile_via_ctypes("/opt/axon/libaxon_pjrt.so")}
        m.set_axon_ntff_profile_hook = lambda h: state.__setitem__("h", h)
        m.get_axon_ntff_profile_hook = lambda: state["h"]
        sys.modules["antenv.axon_hooks"] = m
        antenv.axon_hooks = m
        import concourse.bass_utils as _bu
        _bu.upload_artifacts = lambda tmpdir: tmpdir  # zero-egress container
    except Exception:
        pass


def kernel(**inputs):
    nc = build_program()
    shared = prep_shared(inputs)
    in_maps = []
    for core in range(NCORES):
        m = dict(shared)
        m.update(prep_core(inputs, core))
        in_maps.append(m)
    trace = bool(os.environ.get("BASS_ENCDEC_TRACE"))
    if trace:
        _ensure_ntff_hook()
    res = run_bass_kernel_spmd(nc, in_maps, list(range(NCORES)), trace=trace)
    global LAST_EXEC_NS, LAST_TRACE
    if trace:
        LAST_EXEC_NS = res.exec_time_ns
        LAST_TRACE = res.instructions_and_trace
    outs = []
    for core in range(NCORES):
        o = np.asarray(res.results[core]["out"], dtype=np.float32)
        outs.append(o.reshape(L, BS, V))
    return np.concatenate(outs, axis=1)


if __name__ == "__main__":
    pass

